# revision 18
# baseline (speedup 1.0000x reference)
"""AdaptiveConv (GNN message passing) on 8 TRN2 NeuronCores.

Math (the reference simplifies because gamma*2*(1-lambda) == 1):
    deg  = histogram(col) + 1 ; dinv = rsqrt(deg)
    xh   = dinv * x
    spmm(x)[i] = dinv[i] * ( sum_{e: row_e=i} xh[col_e] + xh[i] )
    for 3 iters:  y = spmm(x); d = y - x0; rn = ||d||_row
                  s = relu(rn - lam) / rn;  x = x0 + s*d

Distribution: nodes row-sharded across 8 cores.  Per iteration:
  1. Iteration 0's gather tables (the AllGathered xh quarters) are
     HOST-precomputed parameters, so the kernel starts gathering
     immediately; iters 1-2 bounce+AllGather per quarter, fired early
     during the previous iteration's rec phase so AG latency hides.
  2. per-edge gather of 256B source rows (gpsimd.dma_gather, int16 idx,
     single_packet=False, calls round-robined over 4 SWDGE queues;
     measured ~2.2ns/descriptor steady-state).  Per pass, dst nodes are
     RANK-SORTED by in-count and packed by water-filling: rank-chunk m
     (128 nodes) gets L_m = max-count chunks; slot q of chunk (m,j) is
     the j-th neighbor of rank 128m+q (ZROW pad).
  3. segment-sum accumulation runs on the TENSOR engine: per (g,j) row
     one fp32r identity matmul accumulates into a PSUM stage tile
     (start at j=0); the Activation engine copies PSUM->SBUF and an SP
     DMA stages to the T_p tables in HBM.  The DVE only does the
     proximal math.
  4. rec: per-position gather of T rows; the 4 passes accumulate in
     PSUM via identity matmuls (2 banks per 16-col tile), and the
     PSUM->SBUF move is fused with the +self-loop add on DVE.
  5. proximal step is node-local vector math, column-tiled so each
     quarter's bounce+AllGather for the NEXT iteration fires as soon as
     its columns are final.

Host-side preprocessing only touches edge_index (graph structure) and
xh0 = dinv*x (cheap numpy).  All iterative compute runs on device fp32.
"""

import math
import numpy as np

import concourse.bass as bass
import concourse.mybir as mybir
import concourse.tile as tile
from concourse import bacc
from concourse.bass_utils import run_bass_kernel_spmd

F32 = mybir.dt.float32
F32R = mybir.dt.float32r
I16 = mybir.dt.int16

CORES = 8
D = 64
K_ITERS = 3
LAMBDA_AMP = 0.1
LAM = (1.0 / (2.0 * (1.0 - LAMBDA_AMP))) * LAMBDA_AMP

GCALL = 32   # chunks per main gather call (4096 idx)
NQ = 4       # SWDGE queues, round-robined


class Plan:
    def __init__(self, N):
        assert N % CORES == 0
        self.N = N
        self.NSH = N // CORES            # 12500
        assert self.NSH % 4 == 0
        self.QP = self.NSH // 4          # 3125
        self.SHQ = self.QP + 3           # + zero pad rows per quarter
        self.SUBT = CORES * self.SHQ     # 25024
        assert self.SUBT <= 32767
        self.CH = int(math.ceil(self.NSH / 128 / 8)) * 8   # 104 cols
        self.NT = 128 * self.CH          # 13312 positions
        self.NRK = int(math.ceil(self.NSH / 128)) * 128    # 12544 ranks
        self.RTILES = self.NT // 1024    # 13 rec tiles
        # rec col-tile ranges (16 cols = 2048 idx per gather, last ragged)
        self.CTILES = []
        c = 0
        while c < self.CH:
            c2 = min(c + 16, self.CH)
            self.CTILES.append((c, c2))
            c = c2
        self.TROWS = (self.NRK // 1024 + 1) * 1024         # 13312 T rows
        self.TZERO = self.TROWS          # index of the zero row


def _wrap16(a):
    """int16 1-D array -> [128, ceil(n/16)] wrapped layout replicated
    across the 8 Q7 core stripes."""
    n = len(a)
    n16 = int(math.ceil(n / 16)) * 16
    b = np.zeros(n16, dtype=np.int16)
    b[:n] = a
    w = b.reshape(-1, 16).T
    return np.ascontiguousarray(np.tile(w, (8, 1)))


def preprocess(x, edge_index):
    N = x.shape[0]
    P = Plan(N)
    NSH, QP, SHQ, CH = P.NSH, P.QP, P.SHQ, P.CH
    ZROW = QP  # first pad row of stripe 0 (zeroed on device)
    row = np.asarray(edge_index[0], dtype=np.int64)
    col = np.asarray(edge_index[1], dtype=np.int64)

    deg = np.bincount(col, minlength=N).astype(np.float64) + 1.0
    dinv_all = (1.0 / np.sqrt(deg)).astype(np.float32)

    # ---- per-core edge lists grouped by (dst, src-quarter) --------------
    cores = []
    for c in range(CORES):
        m = (row >= c * NSH) & (row < (c + 1) * NSH)
        dl = row[m] - c * NSH
        src = col[m]
        lcl = src % NSH
        p_of = lcl // QP
        loc = (src // NSH) * SHQ + (lcl - p_of * QP)
        key = dl * 4 + p_of
        order = np.argsort(key, kind="stable")
        loc_s = loc[order]
        cnt = np.bincount(key, minlength=NSH * 4).reshape(NSH, 4)
        starts = np.concatenate([[0], np.cumsum(cnt.reshape(-1))])[:-1].reshape(NSH, 4)
        # rank per pass: sort nodes by count desc (stable)
        rk_node = []   # rank -> node, padded to NRK
        rk_cnt = []
        for p in range(4):
            o = np.argsort(-cnt[:, p], kind="stable")
            o = np.concatenate([o, np.full(P.NRK - NSH, -1, dtype=np.int64)])
            rk_node.append(o)
            cc = np.where(o >= 0, cnt[np.maximum(o, 0), p], 0)
            rk_cnt.append(cc)
        cores.append({"cnt": cnt, "starts": starts, "loc_s": loc_s,
                      "rk_node": rk_node, "rk_cnt": rk_cnt})

    # ---- global water-fill schedule: L_m = max over cores ---------------
    NM = P.NRK // 128   # 98 rank-chunks per pass
    Lg = np.zeros((4, NM), dtype=np.int64)
    for p in range(4):
        for c in range(CORES):
            Lg[p] = np.maximum(Lg[p], cores[c]["rk_cnt"][p].reshape(NM, 128)[:, 0])
        Lg[p] = np.maximum(Lg[p], 1)
    P.Lg = Lg
    P.cpp = [int(Lg[p].sum()) for p in range(4)]     # chunks per pass
    P.ctot = int(sum(P.cpp))

    # j-major row schedule per pass: within each group of 8 rank-chunks,
    # row (g, j) covers the kj chunks {(8g+mi, j) : Lg[8g+mi] > j} (a
    # prefix, since Lg is non-increasing).  One matmul per row.
    NGRP = (NM + 7) // 8
    P.NGRP = NGRP
    P.rows = []   # per pass: list of (g, j, kj)
    for p in range(4):
        rows_p = []
        for g in range(NGRP):
            msz = min(8, NM - 8 * g)
            Lmax = int(Lg[p][8 * g])
            for j in range(Lmax):
                kj = int(np.sum(Lg[p][8 * g:8 * g + msz] > j))
                rows_p.append((g, j, kj))
        assert sum(k for (_, _, k) in rows_p) == P.cpp[p]
        P.rows.append(rows_p)

    # ---- per-core slot tables + rec index tables ------------------------
    per_core = []
    for c in range(CORES):
        cd = cores[c]
        slots_all = []
        rec_all = []
        for p in range(4):
            rkn, rkc = cd["rk_node"][p], cd["rk_cnt"][p]
            st, ls = cd["starts"], cd["loc_s"]
            slots_p = np.full((P.cpp[p], 128), ZROW, dtype=np.int16)
            ci = 0
            for (g, j, kj) in P.rows[p]:
                for mi in range(kj):
                    mm = 8 * g + mi
                    nodes = rkn[mm * 128:(mm + 1) * 128]
                    cnts = rkc[mm * 128:(mm + 1) * 128]
                    s0 = np.where(nodes >= 0, st[np.maximum(nodes, 0), p], 0)
                    sel = cnts > j
                    slots_p[ci, sel] = ls[s0[sel] + j]
                    ci += 1
            assert ci == P.cpp[p]
            slots_all.append(slots_p.reshape(-1))
            # rec idx: position i = t*1024 + cc*128 + e -> pos (8t+cc)*128+e
            # node at pos (e, ch) is n = e*CH + ch; pos index = ch*128 + e
            rank_of = np.full(NSH, -1, dtype=np.int64)
            valid = rkn >= 0
            rank_of[rkn[valid]] = np.arange(P.NRK)[valid]
            v = np.full(P.NT, P.TZERO, dtype=np.int64)
            n_ids = np.arange(NSH)
            r = rank_of[n_ids]
            trow = (r // 1024) * 1024 + (r % 128) * 8 + (r // 128) % 8
            use = cd["cnt"][:, p] > 0
            v[n_ids[use]] = trow[use]
            rec_all.append(v.astype(np.int16))
        slots_pp = [_wrap16(s) for s in slots_all]
        rec_all = np.concatenate(rec_all)

        # column-major layout: node n <-> (partition n%128, col n//128)
        xt = np.zeros((128 * CH, D), dtype=np.float32)
        xt[:NSH] = x[c * NSH:(c + 1) * NSH]
        dt_ = np.zeros(128 * CH, dtype=np.float32)
        dt_[:NSH] = dinv_all[c * NSH:(c + 1) * NSH]
        xh_ = dt_[:, None] * xt
        def cm(a):          # [128*CH, w] -> [128, CH*w], node n at (n%128, n//128)
            w = a.shape[1] if a.ndim == 2 else 1
            return np.ascontiguousarray(
                a.reshape(CH, 128, w).transpose(1, 0, 2).reshape(128, CH * w))
        per_core.append({
            "x0": cm(xt),
            "xh0": cm(xh_),
            "dinv": cm(dt_[:, None]),
            "slots_main": slots_pp,
            "slots_rec": _wrap16(rec_all),
        })
    P.per_core = per_core

    # ---- iteration-0 gather tables (the AllGathered xh quarters) --------
    xh_full = dinv_all[:, None] * np.asarray(x, dtype=np.float32)
    P.xq0 = []
    for q in range(4):
        t = np.zeros((CORES, SHQ, D), dtype=np.float32)
        for c2 in range(CORES):
            t[c2, :QP] = xh_full[c2 * NSH + q * QP:c2 * NSH + (q + 1) * QP]
        P.xq0.append(np.ascontiguousarray(t.reshape(CORES * SHQ, D)))
    P.ident = np.ascontiguousarray(np.eye(128, dtype=np.float32))
    return P


# ======================================================================
# Bass kernel builder
# ======================================================================

def build_kernel(P: Plan):
    NSH, SUBT, CH, NT = P.NSH, P.SUBT, P.CH, P.NT
    QP, SHQ = P.QP, P.SHQ
    CHD = CH * D
    NM = P.NRK // 128
    TOTR = P.per_core[0]["slots_rec"].shape[1]

    nc = bacc.Bacc(None, target_bir_lowering=False, num_swdge_queues=NQ)

    x0_p = nc.declare_dram_parameter("x0", [128, CHD], F32, isOutput=False)
    xh0_p = nc.declare_dram_parameter("xh0", [128, CHD], F32, isOutput=False)
    dinv_p = nc.declare_dram_parameter("dinv", [128, CH], F32, isOutput=False)
    SMW = [P.per_core[0]["slots_main"][p].shape[1] for p in range(4)]
    sm_p = [nc.declare_dram_parameter(f"slots_main{p}", [128, SMW[p]],
                                      I16, isOutput=False) for p in range(4)]
    sr_p = nc.declare_dram_parameter("slots_rec", [128, TOTR], I16, isOutput=False)
    id_p = nc.declare_dram_parameter("ident", [128, 128], F32R, isOutput=False)
    xq_p = [nc.declare_dram_parameter(f"xq{q}", [SUBT, D], F32R, isOutput=False)
            for q in range(4)]
    out_p = nc.declare_dram_parameter("out", [128, CHD], F32, isOutput=True)

    bounce_q = [nc.dram_tensor(f"bounce{p}", [SHQ, D], F32R) for p in range(4)]
    xh_q = [nc.dram_tensor(f"xhq{p}", [SUBT, D], F32R, addr_space="Shared")
            for p in range(4)]
    tp = [nc.dram_tensor(f"tp{p}", [P.TROWS + 1, D], F32) for p in range(4)]

    qctr = [0]

    def nextq():
        q = qctr[0] % NQ
        qctr[0] += 1
        return q

    with tile.TileContext(nc) as tc:
        with (
            tc.tile_pool(name="persist", bufs=1) as pp,
            tc.tile_pool(name="gmain", bufs=8) as gp,
            tc.tile_pool(name="grec", bufs=4) as grp,
            tc.tile_pool(name="stage", bufs=3) as sp,
            tc.tile_pool(name="stpsum", space="PSUM", bufs=6) as psp,
        ):
            B0 = pp.tile([128, CHD], F32)
            B1 = pp.tile([128, CHD], F32)
            B2 = pp.tile([128, CHD], F32)
            DINV = pp.tile([128, CH], F32)
            SM = [pp.tile([128, SMW[p]], I16, name=f"SMp{p}")
                  for p in range(4)]
            SR = pp.tile([128, TOTR], I16)
            IDT = pp.tile([128, 128], F32R)
            RN = pp.tile([128, CH], F32)
            SC = pp.tile([128, CH], F32)
            RC = pp.tile([128, CH], F32)
            ZT = pp.tile([1, 3 * D], F32)

            # slot tables first: the very first gather only needs SM[0]
            for p in range(4):
                nc.sync.dma_start(out=SM[p][:], in_=sm_p[p][:])
            nc.sync.dma_start(out=IDT[:], in_=id_p[:])
            nc.sync.dma_start(out=B0[:], in_=x0_p[:])
            nc.sync.dma_start(out=DINV[:], in_=dinv_p[:])
            nc.sync.dma_start(out=SR[:], in_=sr_p[:])
            nc.vector.memset(ZT[:], 0.0)
            for p in range(4):
                nc.sync.dma_start(
                    out=bounce_q[p][QP:SHQ, :].rearrange("(o r) f -> o (r f)", o=1),
                    in_=ZT[:1, :3 * D].bitcast(F32R))
                nc.sync.dma_start(
                    out=tp[p][P.TROWS:P.TROWS + 1, :], in_=ZT[:1, :D])

            def dv3(srct, ch8, ncols):
                return srct[:, ch8].rearrange("p (c o) -> p c o", o=1) \
                    .to_broadcast([128, ncols, D])

            def bounce_pieces(q):
                """pieces (rbase, c0, c1, p0, p1) covering nodes
                [q*QP, (q+1)*QP) in column-major layout n=(c*128+p)."""
                pieces = []
                a, b = q * QP, (q + 1) * QP
                base = 0
                if a % 128:
                    c = a // 128
                    take = min(128 - a % 128, b - a)
                    pieces.append((base, c, c + 1, a % 128, a % 128 + take))
                    base += take
                    a += take
                cm0, cm1 = a // 128, b // 128
                if cm1 > cm0:
                    pieces.append((base, cm0, cm1, 0, 128))
                    base += (cm1 - cm0) * 128
                    a = cm1 * 128
                if a < b:
                    pieces.append((base, b // 128, b // 128 + 1, 0, b - a))
                return pieces

            def bounce_and_ag(q, src_t):
                """DMA quarter q of the xh layout into bounce_q[q], then AG.
                src_t is a [128, CH*D] tile in column-major layout."""
                for (rbase, c0, c1, p0, p1) in bounce_pieces(q):
                    n = (c1 - c0) * (p1 - p0)
                    nc.sync.dma_start(
                        out=bounce_q[q][rbase:rbase + n, :]
                        .rearrange("(c p) f -> p c f", p=p1 - p0),
                        in_=src_t[p0:p1, c0 * D:c1 * D]
                        .rearrange("p (c f) -> p c f", f=D).bitcast(F32R),
                    )
                nc.gpsimd.collective_compute(
                    "AllGather",
                    mybir.AluOpType.bypass,
                    replica_groups=[list(range(CORES))],
                    ins=[bounce_q[q][:, :]],
                    outs=[xh_q[q][:, :]],
                )

            # iteration 0's xh tables come precomputed from the host (xq_p);
            # load B1 (self-loop term) from xh0.
            nc.sync.dma_start(out=B1[:], in_=xh0_p[:])

            # quarter q's AG fires at the col-tile where its columns finish
            qfire = {}
            for q in range(4):
                need = -(-((q + 1) * QP) // 128)  # cols needed
                for ti_, (c0_, c1_) in enumerate(P.CTILES):
                    if c1_ >= need:
                        qfire.setdefault(ti_, []).append(q)
                        break

            def emit_proximal(ti, c0, c1, it):
                last_it = (it == K_ITERS - 1)
                ncols = c1 - c0
                cs = slice(c0 * D, c1 * D)
                ch8 = slice(c0, c1)

                def c3(tile_):
                    return tile_[:, cs].rearrange("p (c f) -> p c f", f=D)

                TT = nc.vector.tensor_tensor
                A = mybir.AluOpType
                # + self-loop term
                TT(out=B2[:, cs], in0=B2[:, cs], in1=B1[:, cs], op=A.add)
                TT(out=c3(B2), in0=c3(B2), in1=dv3(DINV, ch8, ncols), op=A.mult)
                TT(out=B1[:, cs], in0=B2[:, cs], in1=B0[:, cs], op=A.subtract)
                TT(out=B2[:, cs], in0=B1[:, cs], in1=B1[:, cs], op=A.mult)
                nc.vector.tensor_reduce(
                    out=RN[:, ch8], in_=c3(B2), axis=mybir.AxisListType.X,
                    op=A.add)
                nc.scalar.sqrt(RN[:, ch8], RN[:, ch8])
                nc.vector.tensor_scalar_add(RC[:, ch8], RN[:, ch8], 1e-30)
                nc.vector.reciprocal(RC[:, ch8], RC[:, ch8])
                nc.vector.tensor_scalar_add(SC[:, ch8], RN[:, ch8], -LAM)
                nc.vector.tensor_scalar_max(SC[:, ch8], SC[:, ch8], 0.0)
                TT(out=SC[:, ch8], in0=SC[:, ch8], in1=RC[:, ch8], op=A.mult)
                TT(out=c3(B1), in0=c3(B1), in1=dv3(SC, ch8, ncols), op=A.mult)
                TT(out=B2[:, cs], in0=B1[:, cs], in1=B0[:, cs], op=A.add)
                if not last_it:
                    TT(out=c3(B1), in0=c3(B2), in1=dv3(DINV, ch8, ncols),
                       op=A.mult)
                    for q in qfire.get(ti, []):
                        bounce_and_ag(q, B1)
                else:
                    nc.sync.dma_start(out=out_p[:, cs], in_=B2[:, cs])

            def emit_rec_tile(rp, ti, c0, c1):
                """One per-position gather tile from tp[rp]; accumulate
                into B2 (Act copy for rp==0, DVE adds after)."""
                ncols = c1 - c0
                g2 = grp.tile([128, 16, D], F32, tag="grec")
                s0 = (rp * NT + c0 * 128) // 16
                nc.gpsimd.dma_gather(
                    g2[:, :ncols, :], tp[rp][:, :],
                    SR[:, s0:s0 + ncols * 8],
                    ncols * 128, ncols * 128, D,
                    elem_step=D,
                    single_packet=False,
                    queue_num=nextq(),
                )
                g2f = g2[:, :ncols, :].rearrange("p c f -> p (c f)")
                cs = slice(c0 * D, c1 * D)
                if rp == 0:
                    nc.scalar.copy(out=B2[:, cs], in_=g2f)
                else:
                    nc.vector.tensor_tensor(
                        out=B2[:, cs], in0=B2[:, cs], in1=g2f,
                        op=mybir.AluOpType.add)

            def emit_rec(rp, it):
                for ti, (c0, c1) in enumerate(P.CTILES):
                    emit_rec_tile(rp, ti, c0, c1)

            for it in range(K_ITERS):
                # ---- main passes: j-major gathers + PE identity accum ----
                # rec tiles of pass p-1 interleave between pass p's gather
                # calls (their staging finished long before), so only rec-3
                # pays a staging-tail wait per iteration.
                chunk0 = 0
                rec_pend = []
                for p in range(4):
                    tbl = xq_p[p] if it == 0 else xh_q[p]
                    cpp = P.cpp[p]
                    rows_p = P.rows[p]
                    # pack rows into gather calls of <= GCALL chunks
                    calls = []   # (chunk_a, chunk_b)
                    ca = 0
                    cc_acc = 0
                    for (g, j, kj) in rows_p:
                        if cc_acc + kj > GCALL:
                            calls.append((ca, ca + cc_acc))
                            ca += cc_acc
                            cc_acc = 0
                        cc_acc += kj
                    if cc_acc:
                        calls.append((ca, ca + cc_acc))
                    gtiles = []
                    for k_call, (a, b) in enumerate(calls):
                        g_t = gp.tile([128, GCALL, D], F32R, tag="gmain")
                        nc.gpsimd.dma_gather(
                            g_t[:, :b - a, :],
                            tbl[:, :],
                            SM[p][:, a * 8:b * 8],
                            (b - a) * 128, (b - a) * 128, D,
                            elem_step=D,
                            single_packet=False,
                            queue_num=nextq(),
                        )
                        gtiles.append((a, g_t))
                        # one pending rec tile per call, from call 3 on
                        # (by then pass p-1's staging tail has drained)
                        if k_call >= 3 and rec_pend:
                            emit_rec_tile(*rec_pend.pop(0))
                    while rec_pend:
                        emit_rec_tile(*rec_pend.pop(0))
                    # one fp32r identity matmul per (g, j) row, accumulating
                    # the j-layers of each rank-chunk group in PSUM
                    ci = 0
                    call_i = 0
                    st_ps = None
                    for (g, j, kj) in rows_p:
                        if call_i + 1 < len(calls) and ci >= calls[call_i][1]:
                            call_i += 1
                        a, g_t = gtiles[call_i]
                        src = g_t[:, ci - a:ci - a + kj, :].rearrange("p c f -> p (c f)")
                        msz = min(8, NM - 8 * g)
                        last = (ci + kj == cpp) or (j + 1 >= int(P.Lg[p][8 * g]))
                        if j == 0:
                            st_ps = psp.tile([128, 512], F32, tag="stps")
                        nc.tensor.matmul(
                            out=st_ps[:, :kj * 64],
                            lhsT=IDT[:],
                            rhs=src,
                            start=(j == 0), stop=last,
                            skip_group_check=True,
                        )
                        ci += kj
                        # group done -> Act copies PSUM->SBUF, DMA stages to T_p
                        if last:
                            st_t = sp.tile([128, 512], F32, tag="stg")
                            nc.scalar.copy(
                                out=st_t[:, :msz * 64], in_=st_ps[:, :msz * 64])
                            if msz == 8:
                                nc.sync.dma_start(
                                    out=tp[p][g * 1024:(g + 1) * 1024, :]
                                    .rearrange("(q cc) f -> q (cc f)", q=128),
                                    in_=st_t[:],
                                )
                            else:
                                nc.sync.dma_start(
                                    out=tp[p][g * 1024:(g + 1) * 1024, :]
                                    .rearrange("(q cc) f -> q cc f", cc=8)[:, :msz, :],
                                    in_=st_t[:, :msz * 64]
                                    .rearrange("q (cc f) -> q cc f", f=D),
                                )
                    assert ci == cpp
                    chunk0 += cpp

                    if p < 3:
                        # defer this pass's rec tiles into the next pass's
                        # gather-call stream
                        rec_pend = [(p, ti, c0, c1)
                                    for ti, (c0, c1) in enumerate(P.CTILES)]
                    else:
                        # last pass: rec right away (one tail wait per iter)
                        emit_rec(3, it)
                # column-tiled proximal + early per-quarter AG
                for ti, (c0, c1) in enumerate(P.CTILES):
                    emit_proximal(ti, c0, c1, it)


    return nc


# ======================================================================
# entry point
# ======================================================================

def _build_and_run(x, edge_index, trace=False):
    x = np.ascontiguousarray(np.asarray(x, dtype=np.float32))
    P = preprocess(x, edge_index)
    nc = build_kernel(P)
    nc.finalize()
    in_maps = []
    for c in range(CORES):
        d = P.per_core[c]
        im = {
            "x0": d["x0"], "xh0": d["xh0"], "dinv": d["dinv"],
            "slots_rec": d["slots_rec"], "ident": P.ident,
        }
        for q in range(4):
            im[f"slots_main{q}"] = d["slots_main"][q]
        for q in range(4):
            im[f"xq{q}"] = P.xq0[q]
        in_maps.append(im)
    res = run_bass_kernel_spmd(nc, in_maps, list(range(CORES)), trace=trace)
    outs = []
    for c in range(CORES):
        o = res.results[c]["out"].reshape(128, P.CH, D) \
            .transpose(1, 0, 2).reshape(128 * P.CH, D)[:P.NSH]
        outs.append(o)
    return np.concatenate(outs, axis=0), res


def kernel(x, edge_index):
    out, _ = _build_and_run(x, edge_index, trace=False)
    return out


# revision 20
# speedup vs baseline: 1.0203x; 1.0203x over previous
"""AdaptiveConv (GNN message passing) on 8 TRN2 NeuronCores.

Math (the reference simplifies because gamma*2*(1-lambda) == 1):
    deg  = histogram(col) + 1 ; dinv = rsqrt(deg)
    xh   = dinv * x
    spmm(x)[i] = dinv[i] * ( sum_{e: row_e=i} xh[col_e] + xh[i] )
    for 3 iters:  y = spmm(x); d = y - x0; rn = ||d||_row
                  s = relu(rn - lam) / rn;  x = x0 + s*d

Distribution: nodes row-sharded across 8 cores.  Per iteration:
  1. Iteration 0's gather tables (the AllGathered xh quarters) are
     HOST-precomputed parameters, so the kernel starts gathering
     immediately; iters 1-2 bounce+AllGather per quarter, fired early
     during the previous iteration's rec phase so AG latency hides.
  2. per-edge gather of 256B source rows (gpsimd.dma_gather, int16 idx,
     single_packet=False, calls round-robined over 4 SWDGE queues;
     measured ~2.2ns/descriptor steady-state).  Per pass, dst nodes are
     RANK-SORTED by in-count and packed by water-filling: rank-chunk m
     (128 nodes) gets L_m = max-count chunks; slot q of chunk (m,j) is
     the j-th neighbor of rank 128m+q (ZROW pad).
  3. segment-sum accumulation runs on the TENSOR engine: per (g,j) row
     one fp32r identity matmul accumulates into a PSUM stage tile
     (start at j=0); the Activation engine copies PSUM->SBUF and an SP
     DMA stages to the T_p tables in HBM.  The DVE only does the
     proximal math.
  4. rec: per-position gather of T rows; the 4 passes accumulate in
     PSUM via identity matmuls (2 banks per 16-col tile), and the
     PSUM->SBUF move is fused with the +self-loop add on DVE.
  5. proximal step is node-local vector math, column-tiled so each
     quarter's bounce+AllGather for the NEXT iteration fires as soon as
     its columns are final.

Host-side preprocessing only touches edge_index (graph structure) and
xh0 = dinv*x (cheap numpy).  All iterative compute runs on device fp32.
"""

import math
import numpy as np

import concourse.bass as bass
import concourse.mybir as mybir
import concourse.tile as tile
from concourse import bacc
from concourse.bass_utils import run_bass_kernel_spmd

F32 = mybir.dt.float32
F32R = mybir.dt.float32r
I16 = mybir.dt.int16

CORES = 8
D = 64
K_ITERS = 3
LAMBDA_AMP = 0.1
LAM = (1.0 / (2.0 * (1.0 - LAMBDA_AMP))) * LAMBDA_AMP

GCALL = 32   # chunks per main gather call (4096 idx)
NQ = 4       # SWDGE queues, round-robined


class Plan:
    def __init__(self, N):
        assert N % CORES == 0
        self.N = N
        self.NSH = N // CORES            # 12500
        assert self.NSH % 4 == 0
        self.QP = self.NSH // 4          # 3125
        self.SHQ = self.QP + 3           # + zero pad rows per quarter
        self.SUBT = CORES * self.SHQ     # 25024
        assert self.SUBT <= 32767
        self.CH = int(math.ceil(self.NSH / 128 / 8)) * 8   # 104 cols
        self.NT = 128 * self.CH          # 13312 positions
        self.NRK = int(math.ceil(self.NSH / 128)) * 128    # 12544 ranks
        self.RTILES = self.NT // 1024    # 13 rec tiles
        # rec col-tile ranges (16 cols = 2048 idx per gather, last ragged)
        self.CTILES = []
        c = 0
        while c < self.CH:
            c2 = min(c + 16, self.CH)
            self.CTILES.append((c, c2))
            c = c2
        self.TROWS = (self.NRK // 1024 + 1) * 1024         # 13312 T rows
        self.TZERO = self.TROWS          # index of the zero row


def _wrap16(a):
    """int16 1-D array -> [128, ceil(n/16)] wrapped layout replicated
    across the 8 Q7 core stripes."""
    n = len(a)
    n16 = int(math.ceil(n / 16)) * 16
    b = np.zeros(n16, dtype=np.int16)
    b[:n] = a
    w = b.reshape(-1, 16).T
    return np.ascontiguousarray(np.tile(w, (8, 1)))


def preprocess(x, edge_index):
    N = x.shape[0]
    P = Plan(N)
    NSH, QP, SHQ, CH = P.NSH, P.QP, P.SHQ, P.CH
    ZROW = QP  # first pad row of stripe 0 (zeroed on device)
    row = np.asarray(edge_index[0], dtype=np.int64)
    col = np.asarray(edge_index[1], dtype=np.int64)

    deg = np.bincount(col, minlength=N).astype(np.float64) + 1.0
    dinv_all = (1.0 / np.sqrt(deg)).astype(np.float32)

    # ---- per-core edge lists grouped by (dst, src-quarter) --------------
    cores = []
    for c in range(CORES):
        m = (row >= c * NSH) & (row < (c + 1) * NSH)
        dl = row[m] - c * NSH
        src = col[m]
        lcl = src % NSH
        p_of = lcl // QP
        loc = (src // NSH) * SHQ + (lcl - p_of * QP)
        key = dl * 4 + p_of
        order = np.argsort(key, kind="stable")
        loc_s = loc[order]
        cnt = np.bincount(key, minlength=NSH * 4).reshape(NSH, 4)
        starts = np.concatenate([[0], np.cumsum(cnt.reshape(-1))])[:-1].reshape(NSH, 4)
        # rank per pass: sort nodes by count desc (stable)
        rk_node = []   # rank -> node, padded to NRK
        rk_cnt = []
        for p in range(4):
            o = np.argsort(-cnt[:, p], kind="stable")
            o = np.concatenate([o, np.full(P.NRK - NSH, -1, dtype=np.int64)])
            rk_node.append(o)
            cc = np.where(o >= 0, cnt[np.maximum(o, 0), p], 0)
            rk_cnt.append(cc)
        cores.append({"cnt": cnt, "starts": starts, "loc_s": loc_s,
                      "rk_node": rk_node, "rk_cnt": rk_cnt})

    # ---- global water-fill schedule: L_m = max over cores ---------------
    NM = P.NRK // 128   # 98 rank-chunks per pass
    Lg = np.zeros((4, NM), dtype=np.int64)
    for p in range(4):
        for c in range(CORES):
            Lg[p] = np.maximum(Lg[p], cores[c]["rk_cnt"][p].reshape(NM, 128)[:, 0])
        Lg[p] = np.maximum(Lg[p], 1)
    P.Lg = Lg
    P.cpp = [int(Lg[p].sum()) for p in range(4)]     # chunks per pass
    P.ctot = int(sum(P.cpp))

    # j-major row schedule per pass: within each group of 8 rank-chunks,
    # row (g, j) covers the kj chunks {(8g+mi, j) : Lg[8g+mi] > j} (a
    # prefix, since Lg is non-increasing).  One matmul per row.
    NGRP = (NM + 7) // 8
    P.NGRP = NGRP
    P.rows = []   # per pass: list of (g, j, kj)
    for p in range(4):
        rows_p = []
        for g in range(NGRP):
            msz = min(8, NM - 8 * g)
            Lmax = int(Lg[p][8 * g])
            for j in range(Lmax):
                kj = int(np.sum(Lg[p][8 * g:8 * g + msz] > j))
                rows_p.append((g, j, kj))
        assert sum(k for (_, _, k) in rows_p) == P.cpp[p]
        P.rows.append(rows_p)

    # ---- per-core slot tables + rec index tables ------------------------
    per_core = []
    for c in range(CORES):
        cd = cores[c]
        slots_all = []
        rec_all = []
        for p in range(4):
            rkn, rkc = cd["rk_node"][p], cd["rk_cnt"][p]
            st, ls = cd["starts"], cd["loc_s"]
            slots_p = np.full((P.cpp[p], 128), ZROW, dtype=np.int16)
            ci = 0
            for (g, j, kj) in P.rows[p]:
                for mi in range(kj):
                    mm = 8 * g + mi
                    nodes = rkn[mm * 128:(mm + 1) * 128]
                    cnts = rkc[mm * 128:(mm + 1) * 128]
                    s0 = np.where(nodes >= 0, st[np.maximum(nodes, 0), p], 0)
                    sel = cnts > j
                    slots_p[ci, sel] = ls[s0[sel] + j]
                    ci += 1
            assert ci == P.cpp[p]
            slots_all.append(slots_p.reshape(-1))
            # rec idx: position i = t*1024 + cc*128 + e -> pos (8t+cc)*128+e
            # node at pos (e, ch) is n = e*CH + ch; pos index = ch*128 + e
            rank_of = np.full(NSH, -1, dtype=np.int64)
            valid = rkn >= 0
            rank_of[rkn[valid]] = np.arange(P.NRK)[valid]
            v = np.full(P.NT, P.TZERO, dtype=np.int64)
            n_ids = np.arange(NSH)
            r = rank_of[n_ids]
            trow = (r // 1024) * 1024 + (r % 128) * 8 + (r // 128) % 8
            use = cd["cnt"][:, p] > 0
            v[n_ids[use]] = trow[use]
            rec_all.append(v.astype(np.int16))
        slots_pp = [_wrap16(s) for s in slots_all]
        rec_all = np.concatenate(rec_all)

        # column-major layout: node n <-> (partition n%128, col n//128)
        xt = np.zeros((128 * CH, D), dtype=np.float32)
        xt[:NSH] = x[c * NSH:(c + 1) * NSH]
        dt_ = np.zeros(128 * CH, dtype=np.float32)
        dt_[:NSH] = dinv_all[c * NSH:(c + 1) * NSH]
        xh_ = dt_[:, None] * xt
        def cm(a):          # [128*CH, w] -> [128, CH*w], node n at (n%128, n//128)
            w = a.shape[1] if a.ndim == 2 else 1
            return np.ascontiguousarray(
                a.reshape(CH, 128, w).transpose(1, 0, 2).reshape(128, CH * w))
        per_core.append({
            "x0": cm(xt),
            "xh0": cm(xh_),
            "dinv": cm(dt_[:, None]),
            "slots_main": slots_pp,
            "slots_rec": _wrap16(rec_all),
        })
    P.per_core = per_core

    # ---- iteration-0 gather tables (the AllGathered xh quarters) --------
    xh_full = dinv_all[:, None] * np.asarray(x, dtype=np.float32)
    P.xq0 = []
    for q in range(4):
        t = np.zeros((CORES, SHQ, D), dtype=np.float32)
        for c2 in range(CORES):
            t[c2, :QP] = xh_full[c2 * NSH + q * QP:c2 * NSH + (q + 1) * QP]
        P.xq0.append(np.ascontiguousarray(t.reshape(CORES * SHQ, D)))
    P.ident = np.ascontiguousarray(np.eye(128, dtype=np.float32))
    return P


# ======================================================================
# Bass kernel builder
# ======================================================================

def build_kernel(P: Plan):
    NSH, SUBT, CH, NT = P.NSH, P.SUBT, P.CH, P.NT
    QP, SHQ = P.QP, P.SHQ
    CHD = CH * D
    NM = P.NRK // 128
    TOTR = P.per_core[0]["slots_rec"].shape[1]

    nc = bacc.Bacc(None, target_bir_lowering=False, num_swdge_queues=NQ)

    x0_p = nc.declare_dram_parameter("x0", [128, CHD], F32, isOutput=False)
    xh0_p = nc.declare_dram_parameter("xh0", [128, CHD], F32, isOutput=False)
    dinv_p = nc.declare_dram_parameter("dinv", [128, CH], F32, isOutput=False)
    SMW = [P.per_core[0]["slots_main"][p].shape[1] for p in range(4)]
    sm_p = [nc.declare_dram_parameter(f"slots_main{p}", [128, SMW[p]],
                                      I16, isOutput=False) for p in range(4)]
    sr_p = nc.declare_dram_parameter("slots_rec", [128, TOTR], I16, isOutput=False)
    id_p = nc.declare_dram_parameter("ident", [128, 128], F32R, isOutput=False)
    xq_p = [nc.declare_dram_parameter(f"xq{q}", [SUBT, D], F32R, isOutput=False)
            for q in range(4)]
    out_p = nc.declare_dram_parameter("out", [128, CHD], F32, isOutput=True)

    bounce_q = [nc.dram_tensor(f"bounce{p}", [SHQ, D], F32R) for p in range(4)]
    xh_q = [nc.dram_tensor(f"xhq{p}", [SUBT, D], F32R, addr_space="Shared")
            for p in range(4)]
    tp = [nc.dram_tensor(f"tp{p}", [P.TROWS + 1, D], F32) for p in range(4)]

    qctr = [0]

    def nextq():
        q = qctr[0] % NQ
        qctr[0] += 1
        return q

    with tile.TileContext(nc) as tc:
        with (
            tc.tile_pool(name="persist", bufs=1) as pp,
            tc.tile_pool(name="gmain", bufs=8) as gp,
            tc.tile_pool(name="grec", bufs=4) as grp,
            tc.tile_pool(name="stage", bufs=3) as sp,
            tc.tile_pool(name="stpsum", space="PSUM", bufs=6) as psp,
        ):
            B0 = pp.tile([128, CHD], F32)
            B1 = pp.tile([128, CHD], F32)
            B2 = pp.tile([128, CHD], F32)
            DINV = pp.tile([128, CH], F32)
            SM = [pp.tile([128, SMW[p]], I16, name=f"SMp{p}")
                  for p in range(4)]
            SR = pp.tile([128, TOTR], I16)
            IDT = pp.tile([128, 128], F32R)
            RN = pp.tile([128, CH], F32)
            SC = pp.tile([128, CH], F32)
            RC = pp.tile([128, CH], F32)
            ZT = pp.tile([1, 3 * D], F32)

            # slot tables first: the very first gather only needs SM[0]
            for p in range(4):
                nc.sync.dma_start(out=SM[p][:], in_=sm_p[p][:])
            nc.sync.dma_start(out=IDT[:], in_=id_p[:])
            nc.sync.dma_start(out=B0[:], in_=x0_p[:])
            nc.sync.dma_start(out=DINV[:], in_=dinv_p[:])
            nc.sync.dma_start(out=SR[:], in_=sr_p[:])
            nc.vector.memset(ZT[:], 0.0)
            for p in range(4):
                nc.sync.dma_start(
                    out=bounce_q[p][QP:SHQ, :].rearrange("(o r) f -> o (r f)", o=1),
                    in_=ZT[:1, :3 * D].bitcast(F32R))
                nc.sync.dma_start(
                    out=tp[p][P.TROWS:P.TROWS + 1, :], in_=ZT[:1, :D])

            def dv3(srct, ch8, ncols):
                return srct[:, ch8].rearrange("p (c o) -> p c o", o=1) \
                    .to_broadcast([128, ncols, D])

            def bounce_pieces(q):
                """pieces (rbase, c0, c1, p0, p1) covering nodes
                [q*QP, (q+1)*QP) in column-major layout n=(c*128+p)."""
                pieces = []
                a, b = q * QP, (q + 1) * QP
                base = 0
                if a % 128:
                    c = a // 128
                    take = min(128 - a % 128, b - a)
                    pieces.append((base, c, c + 1, a % 128, a % 128 + take))
                    base += take
                    a += take
                cm0, cm1 = a // 128, b // 128
                if cm1 > cm0:
                    pieces.append((base, cm0, cm1, 0, 128))
                    base += (cm1 - cm0) * 128
                    a = cm1 * 128
                if a < b:
                    pieces.append((base, b // 128, b // 128 + 1, 0, b - a))
                return pieces

            def bounce_and_ag(q, src_t):
                """DMA quarter q of the xh layout into bounce_q[q], then AG.
                src_t is a [128, CH*D] tile in column-major layout."""
                for (rbase, c0, c1, p0, p1) in bounce_pieces(q):
                    n = (c1 - c0) * (p1 - p0)
                    nc.sync.dma_start(
                        out=bounce_q[q][rbase:rbase + n, :]
                        .rearrange("(c p) f -> p c f", p=p1 - p0),
                        in_=src_t[p0:p1, c0 * D:c1 * D]
                        .rearrange("p (c f) -> p c f", f=D).bitcast(F32R),
                    )
                nc.gpsimd.collective_compute(
                    "AllGather",
                    mybir.AluOpType.bypass,
                    replica_groups=[list(range(CORES))],
                    ins=[bounce_q[q][:, :]],
                    outs=[xh_q[q][:, :]],
                )

            # iteration 0's xh tables come precomputed from the host (xq_p);
            # load B1 (self-loop term) from xh0.
            nc.sync.dma_start(out=B1[:], in_=xh0_p[:])

            # quarter q's AG fires at the col-tile where its columns finish
            qfire = {}
            for q in range(4):
                need = -(-((q + 1) * QP) // 128)  # cols needed
                for ti_, (c0_, c1_) in enumerate(P.CTILES):
                    if c1_ >= need:
                        qfire.setdefault(ti_, []).append(q)
                        break

            def emit_proximal(ti, c0, c1, it):
                last_it = (it == K_ITERS - 1)
                ncols = c1 - c0
                cs = slice(c0 * D, c1 * D)
                ch8 = slice(c0, c1)

                def c3(tile_):
                    return tile_[:, cs].rearrange("p (c f) -> p c f", f=D)

                TT = nc.vector.tensor_tensor
                A = mybir.AluOpType
                # + self-loop term
                TT(out=B2[:, cs], in0=B2[:, cs], in1=B1[:, cs], op=A.add)
                TT(out=c3(B2), in0=c3(B2), in1=dv3(DINV, ch8, ncols), op=A.mult)
                TT(out=B1[:, cs], in0=B2[:, cs], in1=B0[:, cs], op=A.subtract)
                TT(out=B2[:, cs], in0=B1[:, cs], in1=B1[:, cs], op=A.mult)
                nc.vector.tensor_reduce(
                    out=RN[:, ch8], in_=c3(B2), axis=mybir.AxisListType.X,
                    op=A.add)
                nc.scalar.sqrt(RN[:, ch8], RN[:, ch8])
                nc.vector.tensor_scalar_add(RC[:, ch8], RN[:, ch8], 1e-30)
                nc.vector.reciprocal(RC[:, ch8], RC[:, ch8])
                nc.vector.tensor_scalar_add(SC[:, ch8], RN[:, ch8], -LAM)
                nc.vector.tensor_scalar_max(SC[:, ch8], SC[:, ch8], 0.0)
                TT(out=SC[:, ch8], in0=SC[:, ch8], in1=RC[:, ch8], op=A.mult)
                TT(out=c3(B1), in0=c3(B1), in1=dv3(SC, ch8, ncols), op=A.mult)
                TT(out=B2[:, cs], in0=B1[:, cs], in1=B0[:, cs], op=A.add)
                if not last_it:
                    TT(out=c3(B1), in0=c3(B2), in1=dv3(DINV, ch8, ncols),
                       op=A.mult)
                    for q in qfire.get(ti, []):
                        bounce_and_ag(q, B1)
                else:
                    nc.sync.dma_start(out=out_p[:, cs], in_=B2[:, cs])

            def emit_rec(rp, it, fuse_proximal=False):
                """Per-position gathers from tp[rp]; accumulate into B2
                (Act copy for rp==0, DVE adds after).  With fuse_proximal,
                each tile's proximal + early-AG follows its add."""
                qctr[0] = 0
                for ti, (c0, c1) in enumerate(P.CTILES):
                    ncols = c1 - c0
                    g2 = grp.tile([128, 16, D], F32, tag="grec")
                    s0 = (rp * NT + c0 * 128) // 16
                    nc.gpsimd.dma_gather(
                        g2[:, :ncols, :], tp[rp][:, :],
                        SR[:, s0:s0 + ncols * 8],
                        ncols * 128, ncols * 128, D,
                        elem_step=D,
                        single_packet=False,
                        queue_num=nextq(),
                    )
                    g2f = g2[:, :ncols, :].rearrange("p c f -> p (c f)")
                    cs = slice(c0 * D, c1 * D)
                    if rp == 0:
                        nc.scalar.copy(out=B2[:, cs], in_=g2f)
                    else:
                        nc.vector.tensor_tensor(
                            out=B2[:, cs], in0=B2[:, cs], in1=g2f,
                            op=mybir.AluOpType.add)
                    if fuse_proximal:
                        emit_proximal(ti, c0, c1, it)

            for it in range(K_ITERS):
                # ---- main passes: j-major gathers + PE identity accum ----
                chunk0 = 0
                for p in range(4):
                    tbl = xq_p[p] if it == 0 else xh_q[p]
                    cpp = P.cpp[p]
                    rows_p = P.rows[p]
                    # pack rows into gather calls of <= GCALL chunks
                    calls = []   # (chunk_a, chunk_b)
                    ca = 0
                    cc_acc = 0
                    for (g, j, kj) in rows_p:
                        if cc_acc + kj > GCALL:
                            calls.append((ca, ca + cc_acc))
                            ca += cc_acc
                            cc_acc = 0
                        cc_acc += kj
                    if cc_acc:
                        calls.append((ca, ca + cc_acc))
                    gtiles = []
                    qctr[0] = 0
                    for (a, b) in calls:
                        g_t = gp.tile([128, GCALL, D], F32R, tag="gmain")
                        nc.gpsimd.dma_gather(
                            g_t[:, :b - a, :],
                            tbl[:, :],
                            SM[p][:, a * 8:b * 8],
                            (b - a) * 128, (b - a) * 128, D,
                            elem_step=D,
                            single_packet=False,
                            queue_num=nextq(),
                        )
                        gtiles.append((a, g_t))
                    # one fp32r identity matmul per (g, j) row, accumulating
                    # the j-layers of each rank-chunk group in PSUM
                    ci = 0
                    call_i = 0
                    st_ps = None
                    for (g, j, kj) in rows_p:
                        if call_i + 1 < len(calls) and ci >= calls[call_i][1]:
                            call_i += 1
                        a, g_t = gtiles[call_i]
                        src = g_t[:, ci - a:ci - a + kj, :].rearrange("p c f -> p (c f)")
                        msz = min(8, NM - 8 * g)
                        last = (ci + kj == cpp) or (j + 1 >= int(P.Lg[p][8 * g]))
                        if j == 0:
                            st_ps = psp.tile([128, 512], F32, tag="stps")
                        nc.tensor.matmul(
                            out=st_ps[:, :kj * 64],
                            lhsT=IDT[:],
                            rhs=src,
                            start=(j == 0), stop=last,
                            skip_group_check=True,
                        )
                        ci += kj
                        # group done -> Act copies PSUM->SBUF, DMA stages to T_p
                        if last:
                            st_t = sp.tile([128, 512], F32, tag="stg")
                            nc.scalar.copy(
                                out=st_t[:, :msz * 64], in_=st_ps[:, :msz * 64])
                            if msz == 8:
                                nc.sync.dma_start(
                                    out=tp[p][g * 1024:(g + 1) * 1024, :]
                                    .rearrange("(q cc) f -> q (cc f)", q=128),
                                    in_=st_t[:],
                                )
                            else:
                                nc.sync.dma_start(
                                    out=tp[p][g * 1024:(g + 1) * 1024, :]
                                    .rearrange("(q cc) f -> q cc f", cc=8)[:, :msz, :],
                                    in_=st_t[:, :msz * 64]
                                    .rearrange("q (cc f) -> q cc f", f=D),
                                )
                    assert ci == cpp
                    chunk0 += cpp

                    # rec gathers for pass p (tp[p] staged just above);
                    # position-ordered accumulate into B2 overlaps next pass
                    emit_rec(p, it)
                # column-tiled proximal + early per-quarter AG
                for ti, (c0, c1) in enumerate(P.CTILES):
                    emit_proximal(ti, c0, c1, it)


    return nc


# ======================================================================
# entry point
# ======================================================================

def _build_and_run(x, edge_index, trace=False):
    x = np.ascontiguousarray(np.asarray(x, dtype=np.float32))
    P = preprocess(x, edge_index)
    nc = build_kernel(P)
    nc.finalize()
    in_maps = []
    for c in range(CORES):
        d = P.per_core[c]
        im = {
            "x0": d["x0"], "xh0": d["xh0"], "dinv": d["dinv"],
            "slots_rec": d["slots_rec"], "ident": P.ident,
        }
        for q in range(4):
            im[f"slots_main{q}"] = d["slots_main"][q]
        for q in range(4):
            im[f"xq{q}"] = P.xq0[q]
        in_maps.append(im)
    res = run_bass_kernel_spmd(nc, in_maps, list(range(CORES)), trace=trace)
    outs = []
    for c in range(CORES):
        o = res.results[c]["out"].reshape(128, P.CH, D) \
            .transpose(1, 0, 2).reshape(128 * P.CH, D)[:P.NSH]
        outs.append(o)
    return np.concatenate(outs, axis=0), res


def kernel(x, edge_index):
    out, _ = _build_and_run(x, edge_index, trace=False)
    return out
